# revision 3
# baseline (speedup 1.0000x reference)
"""Sliding-window attention kernel for 8 Trainium2 NeuronCores.

Problem: B=1, S=8192, H=3072, 12 heads x 256 head_dim, window 512, stride 448
(overlap 64), overlapping-window attention with overlap-averaged outputs.

Sharding: 4 sequence-groups x 2 head-groups = 8 cores.
 - Each sequence-group owns 5 consecutive windows (19 real windows + 1 dummy),
   i.e. a token span of 448*5+64 = 2304 (zero padded past S).
 - Each head-group owns 6 of the 12 heads (1536 of 3072 feature dims).

Per core (all matmuls bf16 operands, fp32 PSUM accumulation):
 Phase A: QKV projection over the span. Q.T/K.T produced feature-major
   [1536, 2304]; V token-major [2304, 1536]. X.T is SBUF-resident.
 Phase B: per (head, window): scores-transposed S.T[k,q] = K.T^T-less
   matmuls (no on-chip transposes anywhere), masked exp via ScalarE
   (bias = additive key mask, scale = 1/sqrt(hd)), softmax denominator via
   ones-matrix matmul, unnormalized ctx.T = V^T A^T via matmul, then one
   DVE multiply by (1/denominator * overlap_weight). Window-overlap merge
   happens in SBUF via a 64-column carry add; ctx.T lands in DRAM scratch.
 Phase C: out[t, :] = ctx.T^T @ Wo.T via matmuls with resident Wo.T.

Host: slices/transposes/casts inputs per core, computes masks and
overlap-count weights, sums the 8 partial outputs (head-groups add, the
64-token seq-group boundary overlaps add via the weights), adds the
bias row bo + bv @ Wo.T.
"""

import numpy as np
import ml_dtypes

import concourse.bass as bass
import concourse.mybir as mybir
import concourse.tile as tile
from concourse.vector_clock import ScopedClock, VectorClock

P = 128
S = 8192
H = 3072
HEADS = 12
HD = 256
WINDOW = 512
STRIDE = 448
OVERLAP = 64
N_WIN = 19  # ceil(8192/448)
SCALE = 1.0 / np.sqrt(HD)

SEQ_G = 4
HEAD_G = 2
NW = 5                       # windows per seq-group (last group: 4 real + 1 dummy)
SPAN = STRIDE * NW + OVERLAP  # 2304
D_LOC = H // HEAD_G          # 1536
HEADS_LOC = HEADS // HEAD_G  # 6
HC = H // P                  # 24 contraction chunks
DB = D_LOC // P              # 12
TB = SPAN // P               # 18
T_TILES = [512, 512, 512, 512, 256]  # sums to SPAN
OB = H // 512                # 6 output column tiles

BF = mybir.dt.bfloat16
F32 = mybir.dt.float32
AF = mybir.ActivationFunctionType


class PatchedTileContext(tile.TileContext):
    """The walrus build in this container caps per-instruction sync waits
    below what Tile's tail drain aggregates; split them across single-wait
    NoOps executing in order on the SP queue ahead of the drain."""

    def _drain_and_barrier(self, tick_clock, wait_clock):
        gc = tick_clock.global_clock
        for p in range(27):
            if gc[p]:
                partial = VectorClock([gc[i] if i == p else 0 for i in range(27)])
                nop = self.nc.sync.nop()
                wait_clock.add_sem_waits(nop.ins, ScopedClock({None: partial}))
        self.nc.sync.drain()
        self.nc.all_engine_barrier()
        assert self.sems is not None
        popped = self.nc._tile_sem_poison_stack.pop()
        assert popped is self._sem_poison
        self.nc.clear_and_free_semaphores(list(self.sems.allocated().values()))
        self.nc.all_engine_barrier()


def split_excess_waits(nc: bass.Bass) -> int:
    """The walrus build here encodes at most ONE sync-wait per instruction.
    Move excess waits onto same-engine NoOps inserted immediately before the
    carrying instruction (engines execute their stream in order, so this is
    semantically identical)."""
    n_split = 0
    for f in nc.m.functions:
        for bb in f.blocks:
            new_insts = []
            for inst in bb.instructions:
                si = inst.sync_info
                if si is not None and si.on_wait is not None and len(si.on_wait) > 1:
                    waits = list(si.on_wait)
                    for w in waits[:-1]:
                        nop = mybir.InstNoOp(
                            name=nc.get_next_instruction_name(), ins=[], outs=[]
                        )
                        nop.engine = inst.engine
                        nop.sync_info = mybir.SyncInfo(on_wait=[w], on_update=[])
                        nc.register_instruction(nop, overwrite=True)
                        new_insts.append(nop)
                        n_split += 1
                    si.on_wait = [waits[-1]]
                new_insts.append(inst)
            if len(new_insts) != len(bb.instructions):
                bb.instructions[:] = new_insts
    return n_split


def build_nc() -> bass.Bass:
    nc = bass.Bass()
    xT_d = nc.dram_tensor("xT", (H, SPAN), BF, kind="ExternalInput")
    wqT_d = nc.dram_tensor("wqT", (H, D_LOC), BF, kind="ExternalInput")
    wkT_d = nc.dram_tensor("wkT", (H, D_LOC), BF, kind="ExternalInput")
    wvT_d = nc.dram_tensor("wvT", (H, D_LOC), BF, kind="ExternalInput")
    woT_d = nc.dram_tensor("woT", (D_LOC, H), BF, kind="ExternalInput")
    bq_d = nc.dram_tensor("bq", (D_LOC,), F32, kind="ExternalInput")
    bk_d = nc.dram_tensor("bk", (D_LOC,), F32, kind="ExternalInput")
    km_d = nc.dram_tensor("km", (NW, WINDOW), F32, kind="ExternalInput")
    wq_d = nc.dram_tensor("wq", (NW, WINDOW), F32, kind="ExternalInput")
    out_d = nc.dram_tensor("out", (SPAN, H), F32, kind="ExternalOutput")

    with PatchedTileContext(nc) as tc:
        with tc.tile_pool(name="dram", bufs=1, space="DRAM") as dpool:
            qT_s = dpool.tile([D_LOC, SPAN], BF)
            kT_s = dpool.tile([D_LOC, SPAN], BF)
            v_s = dpool.tile([SPAN, D_LOC], BF)
            cT_s = dpool.tile([D_LOC, SPAN], BF)

            # ---------------- Phase A: QKV projections ----------------
            with (
                tc.tile_pool(name="xt", bufs=1) as xtp,
                tc.tile_pool(name="wp", bufs=2) as wpp,
                tc.tile_pool(name="wv", bufs=2) as wvp,
                tc.tile_pool(name="cst", bufs=1) as cstp,
                tc.tile_pool(name="evA", bufs=4) as evp,
                tc.tile_pool(name="psA", bufs=4, space="PSUM") as psA,
            ):
                xt = xtp.tile([P, HC, SPAN], BF)
                nc.sync.dma_start(xt[:], xT_d[:].rearrange("(c p) t -> p c t", p=P))
                bq_sb = cstp.tile([P, DB], F32, tag="bq")
                nc.sync.dma_start(bq_sb[:], bq_d[:].rearrange("(c p) -> p c", p=P))
                bk_sb = cstp.tile([P, DB], F32, tag="bk")
                nc.sync.dma_start(bk_sb[:], bk_d[:].rearrange("(c p) -> p c", p=P))

                for wT_d, scr, bias_sb in (
                    (wqT_d, qT_s, bq_sb),
                    (wkT_d, kT_s, bk_sb),
                ):
                    for db in range(DB):
                        wpan = wpp.tile([P, HC, P], BF, tag="wpan")
                        nc.sync.dma_start(
                            wpan[:],
                            wT_d[:].rearrange("(c p) d -> p c d", p=P)[
                                :, :, db * P:(db + 1) * P
                            ],
                        )
                        t0 = 0
                        for tn in T_TILES:
                            ps = psA.tile([P, 512], F32, tag="psA")
                            for hc in range(HC):
                                nc.tensor.matmul(
                                    ps[:, :tn],
                                    wpan[:, hc, :],
                                    xt[:, hc, t0:t0 + tn],
                                    start=(hc == 0),
                                    stop=(hc == HC - 1),
                                )
                            ev = evp.tile([P, 512], BF, tag="evA")
                            nc.scalar.activation(
                                ev[:, :tn], ps[:, :tn], AF.Identity,
                                bias=bias_sb[:, db:db + 1], scale=1.0,
                            )
                            nc.sync.dma_start(
                                scr[db * P:(db + 1) * P, t0:t0 + tn], ev[:, :tn]
                            )
                            t0 += tn

                for dt3 in range(D_LOC // 512):
                    wvpan = wvp.tile([P, HC, 512], BF, tag="wvpan")
                    nc.sync.dma_start(
                        wvpan[:],
                        wvT_d[:].rearrange("(c p) d -> p c d", p=P)[
                            :, :, dt3 * 512:(dt3 + 1) * 512
                        ],
                    )
                    for tb in range(TB):
                        ps = psA.tile([P, 512], F32, tag="psA")
                        for hc in range(HC):
                            nc.tensor.matmul(
                                ps[:],
                                xt[:, hc, tb * P:(tb + 1) * P],
                                wvpan[:, hc, :],
                                start=(hc == 0),
                                stop=(hc == HC - 1),
                            )
                        ev = evp.tile([P, 512], BF, tag="evA")
                        nc.scalar.activation(ev[:], ps[:], AF.Copy)
                        nc.sync.dma_start(
                            v_s[tb * P:(tb + 1) * P, dt3 * 512:(dt3 + 1) * 512],
                            ev[:],
                        )

            # ---------------- Phase B: windowed attention ----------------
            with (
                tc.tile_pool(name="attc", bufs=1) as attc,
                tc.tile_pool(name="qkv", bufs=3) as qkvp,
                tc.tile_pool(name="ex", bufs=2) as exq,
                tc.tile_pool(name="den", bufs=2) as denp,
                tc.tile_pool(name="cur", bufs=3) as curp,
                tc.tile_pool(name="psS", bufs=1, space="PSUM") as psS,
                tc.tile_pool(name="psM", bufs=1, space="PSUM") as psM,
                tc.tile_pool(name="psX", bufs=1, space="PSUM") as psX,
            ):
                km_sb = attc.tile([P, NW, WINDOW // P], F32, tag="km")
                nc.sync.dma_start(km_sb[:], km_d[:].rearrange("n (c p) -> p n c", p=P))
                wq_sb = attc.tile([P, NW, WINDOW], F32, tag="wq")
                nc.sync.dma_start(
                    wq_sb[:], wq_d[:].unsqueeze(0).to_broadcast((P, NW, WINDOW))
                )
                ones_sb = attc.tile([P, P], BF, tag="ones")
                nc.vector.memset(ones_sb[:], 1.0)

                qT_r = qT_s[:].rearrange("(c p) t -> p c t", p=P)
                kT_r = kT_s[:].rearrange("(c p) t -> p c t", p=P)

                for h in range(HEADS_LOC):
                    prev = None
                    for w in range(NW):
                        w0 = STRIDE * w
                        qts = qkvp.tile([P, 2, WINDOW], BF, tag="qts")
                        nc.sync.dma_start(
                            qts[:], qT_r[:, 2 * h:2 * h + 2, w0:w0 + WINDOW]
                        )
                        kts = qkvp.tile([P, 2, WINDOW], BF, tag="kts")
                        nc.sync.dma_start(
                            kts[:], kT_r[:, 2 * h:2 * h + 2, w0:w0 + WINDOW]
                        )
                        vts = qkvp.tile([P, 4, HD], BF, tag="vts")
                        nc.sync.dma_start(
                            vts[:],
                            v_s[w0:w0 + WINDOW, HD * h:HD * (h + 1)].rearrange(
                                "(c p) d -> p c d", p=P
                            ),
                        )
                        # S.T[k, q] accumulated over the two 128-chunks of hd
                        st = psS.tile([P, 4, WINDOW], F32, tag="st")
                        for kb in range(4):
                            for dc in range(2):
                                nc.tensor.matmul(
                                    st[:, kb, :],
                                    kts[:, dc, kb * P:(kb + 1) * P],
                                    qts[:, dc, :],
                                    start=(dc == 0),
                                    stop=(dc == 1),
                                )
                        ex = exq.tile([P, 4, WINDOW], BF, tag="ex")
                        for kb in range(4):
                            nc.scalar.activation(
                                ex[:, kb, :], st[:, kb, :], AF.Exp,
                                bias=km_sb[:, w, kb:kb + 1], scale=float(SCALE),
                            )
                        # denominator: every partition gets the column sum
                        sm = psM.tile([P, WINDOW], F32, tag="sm")
                        for kb in range(4):
                            nc.tensor.matmul(
                                sm[:], ones_sb[:], ex[:, kb, :],
                                start=(kb == 0), stop=(kb == 3),
                            )
                        den = denp.tile([P, WINDOW], F32, tag="den")
                        nc.vector.reciprocal(den[:], sm[:])
                        nc.vector.tensor_mul(
                            out=den[:], in0=den[:], in1=wq_sb[:, w, :]
                        )
                        # ctx.T[d, q] unnormalized
                        ctxp = psX.tile([P, 2, WINDOW], F32, tag="ctxp")
                        for db2 in range(2):
                            for kb in range(4):
                                nc.tensor.matmul(
                                    ctxp[:, db2, :],
                                    vts[:, kb, db2 * P:(db2 + 1) * P],
                                    ex[:, kb, :],
                                    start=(kb == 0),
                                    stop=(kb == 3),
                                )
                        cur = curp.tile([P, 2, WINDOW], BF, tag="cur")
                        for db2 in range(2):
                            nc.vector.tensor_mul(
                                out=cur[:, db2, :], in0=ctxp[:, db2, :], in1=den[:]
                            )
                        if prev is not None:
                            for db2 in range(2):
                                nc.vector.tensor_add(
                                    out=cur[:, db2, 0:OVERLAP],
                                    in0=cur[:, db2, 0:OVERLAP],
                                    in1=prev[:, db2, STRIDE:WINDOW],
                                )
                        for db2 in range(2):
                            r0 = HD * h + P * db2
                            nc.sync.dma_start(
                                cT_s[r0:r0 + P, w0:w0 + STRIDE],
                                cur[:, db2, 0:STRIDE],
                            )
                        if w == NW - 1:
                            for db2 in range(2):
                                r0 = HD * h + P * db2
                                nc.sync.dma_start(
                                    cT_s[r0:r0 + P, w0 + STRIDE:w0 + WINDOW],
                                    cur[:, db2, STRIDE:WINDOW],
                                )
                        prev = cur

            # ---------------- Phase C: output projection ----------------
            with (
                tc.tile_pool(name="wo", bufs=1) as wop,
                tc.tile_pool(name="cpan", bufs=3) as cpp,
                tc.tile_pool(name="oev", bufs=4) as oevp,
                tc.tile_pool(name="psO", bufs=4, space="PSUM") as psO,
            ):
                wo_sb = wop.tile([P, DB, H], BF)
                nc.sync.dma_start(
                    wo_sb[:], woT_d[:].rearrange("(c p) o -> p c o", p=P)
                )
                cT_r = cT_s[:].rearrange("(c p) t -> p c t", p=P)
                for tb in range(TB):
                    cpan = cpp.tile([P, DB, P], BF, tag="cpan")
                    nc.sync.dma_start(cpan[:], cT_r[:, :, tb * P:(tb + 1) * P])
                    for ob in range(OB):
                        ps = psO.tile([P, 512], F32, tag="psO")
                        for dc in range(DB):
                            nc.tensor.matmul(
                                ps[:],
                                cpan[:, dc, :],
                                wo_sb[:, dc, ob * 512:(ob + 1) * 512],
                                start=(dc == 0),
                                stop=(dc == DB - 1),
                            )
                        ev = oevp.tile([P, 512], F32, tag="oev")
                        nc.scalar.activation(ev[:], ps[:], AF.Copy)
                        nc.sync.dma_start(
                            out_d[tb * P:(tb + 1) * P, ob * 512:(ob + 1) * 512],
                            ev[:],
                        )
    split_excess_waits(nc)
    return nc


def make_in_maps(inputs: dict) -> list[dict]:
    """Shard the full problem inputs into 8 per-core input maps."""
    X = np.asarray(inputs["hidden_states"], dtype=np.float32)[0]     # [S, H]
    Wq = np.asarray(inputs["Wq"], dtype=np.float32)
    Wk = np.asarray(inputs["Wk"], dtype=np.float32)
    Wv = np.asarray(inputs["Wv"], dtype=np.float32)
    Wo = np.asarray(inputs["Wo"], dtype=np.float32)
    bq = np.asarray(inputs["bq"], dtype=np.float32)
    bk = np.asarray(inputs["bk"], dtype=np.float32)

    counts = np.zeros(S, np.float32)
    for w in range(N_WIN):
        counts[STRIDE * w: min(STRIDE * w + WINDOW, S)] += 1.0

    bf = ml_dtypes.bfloat16
    wT = {}
    for hg in range(HEAD_G):
        sl = slice(D_LOC * hg, D_LOC * (hg + 1))
        wT[hg] = {
            "wqT": np.ascontiguousarray(Wq[sl, :].T.astype(bf)),
            "wkT": np.ascontiguousarray(Wk[sl, :].T.astype(bf)),
            "wvT": np.ascontiguousarray(Wv[sl, :].T.astype(bf)),
            "woT": np.ascontiguousarray(Wo[:, sl].T.astype(bf)),
            "bq": np.ascontiguousarray(bq[sl]),
            "bk": np.ascontiguousarray(bk[sl]),
        }

    per_g = {}
    for g in range(SEQ_G):
        s0 = STRIDE * NW * g
        xs = np.zeros((SPAN, H), np.float32)
        seg = X[s0: min(s0 + SPAN, S)]
        xs[: seg.shape[0]] = seg
        km = np.zeros((NW, WINDOW), np.float32)
        wq = np.zeros((NW, WINDOW), np.float32)
        pos = np.arange(WINDOW)
        for wl in range(NW):
            wg = NW * g + wl
            if wg >= N_WIN:
                continue  # dummy window: km stays 0 (finite), wq stays 0
            tok = STRIDE * wg + pos
            km[wl] = np.where(tok < S, 0.0, -1e9)
            wq[wl] = np.where(tok < S, 1.0 / np.maximum(counts[np.minimum(tok, S - 1)], 1.0), 0.0)
            wq[wl][tok >= S] = 0.0
        per_g[g] = {
            "xT": np.ascontiguousarray(xs.T.astype(bf)),
            "km": km,
            "wq": wq,
        }

    in_maps = []
    for g in range(SEQ_G):
        for hg in range(HEAD_G):
            m = {}
            m.update(per_g[g])
            m.update(wT[hg])
            in_maps.append(m)
    return in_maps


def combine(results: list[dict], inputs: dict) -> np.ndarray:
    Wo = np.asarray(inputs["Wo"], dtype=np.float32)
    bv = np.asarray(inputs["bv"], dtype=np.float32)
    bo = np.asarray(inputs["bo"], dtype=np.float32)
    out = np.zeros((S, H), np.float32)
    for g in range(SEQ_G):
        s0 = STRIDE * NW * g
        L = min(SPAN, S - s0)
        for hg in range(HEAD_G):
            out[s0:s0 + L] += results[g * HEAD_G + hg]["out"][:L]
    out += (bo + Wo @ bv)[None, :]
    return out[None]


_NC_CACHE: dict = {}


def kernel(**inputs) -> np.ndarray:
    if "nc" not in _NC_CACHE:
        _NC_CACHE["nc"] = build_nc()
    nc = _NC_CACHE["nc"]
    in_maps = make_in_maps(inputs)
    from concourse.bass_utils import run_bass_kernel_spmd
    res = run_bass_kernel_spmd(nc, in_maps, core_ids=list(range(8)))
    return combine(res.results, inputs).astype(np.float32)
